# revision 1
# baseline (speedup 1.0000x reference)
"""Cross-covariance attention (XCA) Trainium2 kernel.

Reference (per batch element b of 8, one NeuronCore each):
    qkv = x @ W_qkv                                  # [n, 3c]
    q, k, v -> per head h: Q_h, K_h, V_h             # [n, d] columns of qkv
    attn_h = softmax_e( (Q_h^T K_h) * t_h / (|q_d| |k_e|) )   # [d, d]
    out_h = attn_h @ V_h^T                           # [d, n]
    y = concat_h(out_h)^T @ W_proj + b_proj          # [n, c]

Kernel strategy (all matmuls fp32r = full-rate relaxed fp32; Grams bf16):
  P1: x^T via PE transposes (fp32, exact)
  P2: stream QK = x @ W_qk per 128-token tile; fuse per-head Gram
      accumulation (bf16, PSUM-resident) and column-norm^2 accumulation
  P3: norms via ones-matmuls; softmax on [96, 96] tiles; A^T via PE transpose
  P3.5: M_h = W_v_h @ A_h^T  (folds V projection and attention together)
  P4: per 256-token chunk: OXT_h = M_h^T @ x^T chunk; y = OXT^T @ W_proj + b
"""
import sys

sys.path.insert(0, "/opt/trn_rl_repo")

import numpy as np
import bass_rust
import concourse.bass as bass
import concourse.mybir as mybir
from concourse.tile import TileContext
from concourse.bass_utils import run_bass_kernel_spmd
from concourse.masks import make_identity
from contextlib import ExitStack

F32 = mybir.dt.float32
F32R = mybir.dt.float32r
BF16 = mybir.dt.bfloat16
AF = mybir.ActivationFunctionType
ALU = mybir.AluOpType
AX = mybir.AxisListType

P = 128
NTOK = 4096
C = 768
H = 8
D = 96
KT = C // P            # 6 contraction tiles over c
NT = NTOK // P         # 32 token tiles
CH = 256               # phase-4 token chunk
NCH = NTOK // CH       # 16 chunks
EPS = 1e-12
N_CORES = 8


def split_multi_waits(nc):
    """This neuronxcc build accepts only ONE sync-wait command per TPB
    instruction; Tile's wait-assignment can attach several. Hoist extras onto
    single-wait NoOps inserted just before, on the same engine."""
    for f in nc.m.functions:
        for blk in f.blocks:
            il = blk.instructions
            i = 0
            while i < len(il):
                inst = il[i]
                si = inst.sync_info
                if si is not None and len(si.on_wait) > 1:
                    waits = list(si.on_wait)
                    inst.sync_info = bass_rust.SyncInfo(
                        on_wait=[waits[-1]], on_update=list(si.on_update)
                    )
                    for j, w in enumerate(waits[:-1]):
                        nop = mybir.InstNoOp(name=f"{inst.name}-sw{j}", ins=[], outs=[])
                        nop.engine = inst.engine
                        nop.sync_info = bass_rust.SyncInfo(on_wait=[w], on_update=[])
                        il.insert(i + j, nop)
                    i += len(waits) - 1
                i += 1


def build_full(debug=False):
    nc = bass.Bass()
    x = nc.declare_dram_parameter("x", [NTOK, C], F32, isOutput=False)
    wqkv = nc.declare_dram_parameter("w_qkv", [C, 3 * C], F32, isOutput=False)
    wproj = nc.declare_dram_parameter("w_proj", [C, C], F32, isOutput=False)
    bproj = nc.declare_dram_parameter("b_proj", [1, C], F32, isOutput=False)
    temp = nc.declare_dram_parameter("temperature", [1, H], F32, isOutput=False)
    y = nc.declare_dram_parameter("y", [NTOK, C], F32, isOutput=True)
    if debug:
        dbg_qk = nc.declare_dram_parameter("dbg_qk", [P, 2 * C], F32, isOutput=True)
        dbg_sq = nc.declare_dram_parameter("dbg_sq", [P, 2 * C], F32, isOutput=True)
        dbg_s = nc.declare_dram_parameter("dbg_s", [D, 2 * 4 * D], F32, isOutput=True)
        dbg_at = nc.declare_dram_parameter("dbg_at", [D, H * D], F32, isOutput=True)
        dbg_m = nc.declare_dram_parameter("dbg_m", [P, KT * C], F32, isOutput=True)
        dbg_oxt = nc.declare_dram_parameter("dbg_oxt", [D, H * CH], F32, isOutput=True)

    with TileContext(nc) as tc, ExitStack() as ctx:
        pers = ctx.enter_context(tc.tile_pool(name="pers", bufs=1))
        ident = pers.tile([P, P], F32)
        make_identity(nc, ident[:])
        ones_col = pers.tile([P, 1], F32)
        nc.vector.memset(ones_col[:], 1.0)
        ones_row = pers.tile([1, P], F32)
        nc.vector.memset(ones_row[:], 1.0)
        temp_sb = pers.tile([1, H], F32)
        nc.sync.dma_start(out=temp_sb[:], in_=temp[:, :])
        xT = pers.tile([P, KT * NTOK], F32R)
        wv = pers.tile([P, KT * C], F32)
        atall = pers.tile([D, H * D], F32R)

        for k in range(KT):
            nc.sync.dma_start(out=wv[:, k * C:(k + 1) * C],
                              in_=wqkv[k * P:(k + 1) * P, 2 * C:3 * C])

        # ======== phases 1-3 in a closeable SBUF scope ========
        with tc.tile_pool(name="p2", bufs=1) as p2:
            wqk = p2.tile([P, KT * 2 * C], F32R)
            for k in range(KT):
                wtmp = p2.tile([P, 2 * C], F32, tag="wtmp", bufs=2, name=f"wtmp{k}")
                nc.sync.dma_start(out=wtmp[:], in_=wqkv[k * P:(k + 1) * P, 0:2 * C])
                nc.scalar.copy(wqk[:, k * 2 * C:(k + 1) * 2 * C], wtmp[:])
            SQ = p2.tile([P, 2 * C], F32)
            nc.vector.memset(SQ[:], 0.0)

            with tc.tile_pool(name="psS", bufs=1, space="PSUM") as psS:
                S_ps = [psS.tile([D, 4 * D], F32, name="S0"),
                        psS.tile([D, 4 * D], F32, name="S1")]

                # ---- phase 1 ----
                with tc.tile_pool(name="p1", bufs=1) as p1, \
                     tc.tile_pool(name="p1ps", bufs=1, space="PSUM") as p1ps:
                    for m in range(NT):
                        xl = p1.tile([P, C], F32, tag="xl", bufs=3, name=f"xl{m}")
                        nc.sync.dma_start(out=xl[:], in_=x[m * P:(m + 1) * P, :])
                        for k in range(KT):
                            tp = p1ps.tile([P, P], F32, tag="tp", bufs=4,
                                           name=f"tp{m}_{k}")
                            nc.tensor.transpose(tp[:], xl[:, k * P:(k + 1) * P],
                                                ident[:])
                            nc.scalar.copy(
                                xT[:, k * NTOK + m * P:k * NTOK + (m + 1) * P], tp[:])

                # ---- phase 2 ----
                qk_ring = []
                with tc.tile_pool(name="psqk", bufs=1, space="PSUM") as psqk:
                    def grams(j):
                        # start=True clears has_written bits for the WHOLE
                        # psum bank, so only the first region per bank may
                        # issue it; the other regions' first write then lands
                        # in overwrite mode (bits cleared by that same start).
                        # tile_critical pins the in-bank emission order.
                        ring = qk_ring[j % 4]

                        def emit():
                            for h in range(H):
                                nc.tensor.matmul(
                                    S_ps[h // 4][:, (h % 4) * D:(h % 4 + 1) * D],
                                    ring[:, h * D:(h + 1) * D],
                                    ring[:, C + h * D:C + (h + 1) * D],
                                    start=(j == 0 and h % 4 == 0),
                                    stop=(j == NT - 1),
                                    skip_group_check=True,
                                )

                        if j == 0:
                            with tc.tile_critical():
                                emit()
                        else:
                            emit()

                    for m in range(NT):
                        if len(qk_ring) < 4:
                            ring = p2.tile([P, 2 * C], BF16, tag="qkring", bufs=4,
                                           name=f"qkring{m}")
                            qk_ring.append(ring)
                        else:
                            ring = qk_ring[m % 4]
                        for chn in range(3):
                            ps = psqk.tile([P, 512], F32, tag="qk", bufs=5,
                                           name=f"qkps{m}_{chn}")
                            for k in range(KT):
                                nc.tensor.matmul(
                                    ps[:],
                                    xT[:, k * NTOK + m * P:k * NTOK + (m + 1) * P],
                                    wqk[:, k * 2 * C + chn * 512:
                                        k * 2 * C + (chn + 1) * 512],
                                    start=(k == 0), stop=(k == KT - 1),
                                )
                            nc.scalar.copy(ring[:, chn * 512:(chn + 1) * 512], ps[:])
                            sqt = p2.tile([P, 512], F32, tag="sqtmp", bufs=1,
                                          name=f"sqt{m}_{chn}")
                            nc.scalar.square(sqt[:], ps[:])
                            sl = SQ[:, chn * 512:(chn + 1) * 512]
                            nc.vector.tensor_add(sl, sl, sqt[:])
                        if m > 0:
                            grams(m - 1)
                    grams(NT - 1)
                    if debug:
                        dqk = p2.tile([P, 2 * C], F32, tag="wtmp", bufs=2,
                                      name="dqk")
                        nc.vector.tensor_copy(dqk[:], qk_ring[0][:])
                        nc.sync.dma_start(out=dbg_qk[:, :], in_=dqk[:])
                        nc.sync.dma_start(out=dbg_sq[:, :], in_=SQ[:])
                        dstile = p2.tile([D, 8 * D], F32, tag="wtmp", bufs=2,
                                         name="dstile")
                        nc.scalar.copy(dstile[:, 0:4 * D], S_ps[0][:])
                        nc.scalar.copy(dstile[:, 4 * D:8 * D], S_ps[1][:])
                        nc.sync.dma_start(out=dbg_s[:, :], in_=dstile[:])

                # ---- phase 3 ----
                with tc.tile_pool(name="p3ps", bufs=1, space="PSUM") as p3ps:
                    rq2 = p3ps.tile([D, H], F32, tag="misc", bufs=4)
                    for h in range(H):
                        nc.tensor.matmul(rq2[:, h:h + 1], SQ[:, h * D:(h + 1) * D],
                                         ones_col[:], start=True, stop=True)
                    rq_sb = p2.tile([D, H], F32)
                    nc.scalar.sqrt(rq_sb[:], rq2[:])
                    nc.vector.tensor_scalar_max(rq_sb[:], rq_sb[:], EPS)
                    nc.vector.reciprocal(rq_sb[:], rq_sb[:])

                    rk_sb = p2.tile([1, C], F32)
                    for i in range(2):
                        nk2 = p3ps.tile([1, 384], F32, tag="misc", bufs=4,
                                        name=f"nk2_{i}")
                        nc.tensor.matmul(nk2[:], ones_col[:],
                                         SQ[:, C + i * 384:C + (i + 1) * 384],
                                         start=True, stop=True)
                        nc.scalar.sqrt(rk_sb[:, i * 384:(i + 1) * 384], nk2[:])
                    nc.vector.tensor_scalar_max(rk_sb[:], rk_sb[:], EPS)
                    nc.vector.reciprocal(rk_sb[:], rk_sb[:])
                    for h in range(H):
                        sl = rk_sb[:, h * D:(h + 1) * D]
                        nc.vector.tensor_scalar(sl, sl, temp_sb[0:1, h:h + 1],
                                                None, ALU.mult)

                    rkb_sb = p2.tile([D, C], F32)
                    for i in range(2):
                        rkb = p3ps.tile([D, 384], F32, tag="misc", bufs=4,
                                        name=f"rkb_{i}")
                        for hh in range(4):
                            h = i * 4 + hh
                            nc.tensor.matmul(rkb[:, hh * D:(hh + 1) * D],
                                             ones_row[0:1, 0:D],
                                             rk_sb[0:1, h * D:(h + 1) * D],
                                             start=True, stop=True)
                        nc.scalar.copy(rkb_sb[:, i * 384:(i + 1) * 384], rkb[:])

                    for h in range(H):
                        Ssl = S_ps[h // 4][:, (h % 4) * D:(h % 4 + 1) * D]
                        L = p2.tile([D, D], F32, tag="L", bufs=2, name=f"L{h}")
                        nc.vector.scalar_tensor_tensor(
                            L[:], Ssl, rq_sb[:, h:h + 1],
                            rkb_sb[:, h * D:(h + 1) * D], ALU.mult, ALU.mult)
                        negmax = p2.tile([D, 1], F32, tag="negmax", bufs=2,
                                         name=f"nm{h}")
                        nc.vector.tensor_reduce(out=negmax[:], in_=L[:], op=ALU.max,
                                                axis=AX.X, negate=True)
                        E = p2.tile([D, D], F32, tag="E", bufs=2, name=f"E{h}")
                        Z = p2.tile([D, 1], F32, tag="Z", bufs=2, name=f"Z{h}")
                        nc.scalar.activation(E[:], L[:], AF.Exp, bias=negmax[:],
                                             scale=1.0, accum_out=Z[:])
                        nc.vector.reciprocal(Z[:], Z[:])
                        A = p2.tile([D, D], F32, tag="A", bufs=2, name=f"A{h}")
                        nc.vector.tensor_scalar(A[:], E[:], Z[:], None, ALU.mult)
                        atp = p3ps.tile([D, D], F32, tag="misc", bufs=4,
                                        name=f"atp{h}")
                        nc.tensor.transpose(atp[:], A[:], ident[0:D, 0:D])
                        nc.scalar.copy(atall[:, h * D:(h + 1) * D], atp[:])
        # p2 / psS closed here

        # ======== phase 3.5 + 4 ========
        with tc.tile_pool(name="p4", bufs=1) as p4, \
             tc.tile_pool(name="p4ps", bufs=1, space="PSUM") as p4ps:
            # M_h = W_v_h @ A_h^T, laid out [128, ct*C + h*D + d]
            M_sb = p4.tile([P, KT * C], F32R)
            for h in range(H):
                wvt = p4.tile([D, C], F32R, tag="wvth", bufs=2, name=f"wvt{h}")
                for ct in range(KT):
                    wtp = p4ps.tile([D, P], F32, tag="wvtp", bufs=2,
                                    name=f"wtp{h}_{ct}")
                    nc.tensor.transpose(wtp[:], wv[:, ct * C + h * D:
                                                   ct * C + (h + 1) * D],
                                        ident[:])
                    nc.scalar.copy(wvt[:, ct * P:(ct + 1) * P], wtp[:])
                for ct in range(KT):
                    mp = p4ps.tile([P, D], F32, tag="mps", bufs=2,
                                   name=f"mp{h}_{ct}")
                    nc.tensor.matmul(mp[:], wvt[:, ct * P:(ct + 1) * P],
                                     atall[:, h * D:(h + 1) * D],
                                     start=True, stop=True)
                    nc.scalar.copy(M_sb[:, ct * C + h * D:ct * C + (h + 1) * D],
                                   mp[:])

            # W_proj head-split rows, rounded to f32r; bias broadcast
            wpr = p4.tile([D, H * C], F32R)
            for h in range(H):
                wptmp = p4.tile([D, C], F32, tag="wptmp", bufs=1, name=f"wptmp{h}")
                nc.sync.dma_start(out=wptmp[:], in_=wproj[h * D:(h + 1) * D, :])
                nc.scalar.copy(wpr[:, h * C:(h + 1) * C], wptmp[:])
            brow = p4.tile([1, C], F32)
            nc.sync.dma_start(out=brow[:], in_=bproj[:, :])
            bias_sb = p4.tile([P, C], F32)
            for i in range(2):
                bp = p4ps.tile([P, 384], F32, tag="proj", bufs=2, name=f"bp{i}")
                nc.tensor.matmul(bp[:], ones_row[:],
                                 brow[0:1, i * 384:(i + 1) * 384],
                                 start=True, stop=True)
                nc.scalar.copy(bias_sb[:, i * 384:(i + 1) * 384], bp[:])

            oxt_tiles = {}

            def oxt_chunk(c):
                ox = p4.tile([D, H * CH], F32R, tag="oxt", bufs=3, name=f"oxt{c}")
                oxt_tiles[c] = ox
                for h in range(H):
                    op = p4ps.tile([D, CH], F32, tag="oxtps", bufs=2,
                                   name=f"oxp{c}_{h}")
                    for ct in range(KT):
                        nc.tensor.matmul(
                            op[:],
                            M_sb[:, ct * C + h * D:ct * C + (h + 1) * D],
                            xT[:, ct * NTOK + c * CH:ct * NTOK + (c + 1) * CH],
                            start=(ct == 0), stop=(ct == KT - 1),
                        )
                    nc.scalar.copy(ox[:, h * CH:(h + 1) * CH], op[:])

            def proj_chunk(c):
                ox = oxt_tiles.pop(c)
                for mt in range(CH // P):
                    fin = p4.tile([P, C], F32, tag="fin", bufs=2,
                                  name=f"fin{c}_{mt}")
                    for i in range(2):
                        pp = p4ps.tile([P, 384], F32, tag="proj", bufs=2,
                                       name=f"pp{c}_{mt}_{i}")
                        for h in range(H):
                            nc.tensor.matmul(
                                pp[:],
                                ox[:, h * CH + mt * P:h * CH + (mt + 1) * P],
                                wpr[:, h * C + i * 384:h * C + (i + 1) * 384],
                                start=(h == 0), stop=(h == H - 1),
                            )
                        nc.vector.scalar_tensor_tensor(
                            fin[:, i * 384:(i + 1) * 384], pp[:], 1.0,
                            bias_sb[:, i * 384:(i + 1) * 384], ALU.mult, ALU.add)
                    nc.sync.dma_start(out=y[c * CH + mt * P:c * CH + (mt + 1) * P, :],
                                      in_=fin[:])

            if debug:
                nc.sync.dma_start(out=dbg_at[:, :], in_=atall[:].bitcast(F32))
                nc.sync.dma_start(out=dbg_m[:, :], in_=M_sb[:].bitcast(F32))
            oxt_chunk(0)
            if debug:
                nc.sync.dma_start(out=dbg_oxt[:, :],
                                  in_=oxt_tiles[0][:].bitcast(F32))
            for c in range(1, NCH):
                oxt_chunk(c)
                proj_chunk(c - 1)
            proj_chunk(NCH - 1)

    split_multi_waits(nc)
    return nc


_PROGRAM = None


def _get_program():
    global _PROGRAM
    if _PROGRAM is None:
        _PROGRAM = build_full()
    return _PROGRAM


def kernel(x, W_qkv, W_proj, b_proj, temperature):
    x = np.asarray(x, dtype=np.float32)
    W_qkv = np.asarray(W_qkv, dtype=np.float32)
    W_proj = np.asarray(W_proj, dtype=np.float32)
    b_proj = np.asarray(b_proj, dtype=np.float32).reshape(1, C)
    temperature = np.asarray(temperature, dtype=np.float32).reshape(1, H)
    b = x.shape[0]
    assert b == N_CORES

    nc = _get_program()
    in_maps = [
        {
            "x": np.ascontiguousarray(x[i]),
            "w_qkv": W_qkv,
            "w_proj": W_proj,
            "b_proj": b_proj,
            "temperature": temperature,
        }
        for i in range(b)
    ]
    res = run_bass_kernel_spmd(nc, in_maps, core_ids=list(range(N_CORES)))
    out = np.stack([res.results[i]["y"] for i in range(N_CORES)], axis=0)
    return out.astype(np.float32)

